# revision 63
# baseline (speedup 1.0000x reference)
"""CapsNet dynamic-routing kernel for 8 Trainium2 NeuronCores (v3).

Sharding: tensor-parallel over N_OUT (8 output capsules per core). x_hat is
never materialized; contractions are re-expressed against W and x:

  s_t[b,o,do]     = sum_{i,di} W[o,i,do,di] * c_t[b,o,i] * x[b,i,di]
  beta_inc[b,o,i] = sum_di ( sum_do v[b,o,do] W[o,i,do,di] ) * x[b,i,di]

Per routing iteration:
  1. Wv matmuls into a rotating 4-quarter PSUM tile; [128,1024] pair-drains:
     ACT copy with the squash factor f as per-partition scale (+ DVE mul by
     x), or fused DVE STT (psum*f)*x.
  2. di-major tree-adds -> beta increment (fast 16-bit DVE path).
  3. beta transposed on PE (fp32; it2 accumulates beta1^T + binc2^T in PSUM);
     ACT-Exp drains the transposed PSUM -> eT[i, h, ic, b] directly: no
     separate exp pass, no per-o e transposes, no Z select-matmul, no
     post-AR rz transposes.
  4. ZT = sum_o eT via 7 contiguous DVE adds; single bf16 AllReduce in the
     transposed layout (a tiny dummy collective during iteration 0 absorbs
     the one-time mesh-setup cost).
  5. y_o = eT_o (bcast over di) * xz, xz = xt * rzT; s-matmuls o-pair
     interleaved for PE col-tile concurrency (at most one open accumulation
     chain per PE col-group: deeper interleave clobbers in-flight weights).
  6. squash kept as [b,o]-scalar factors f (fT2), folded into drains and the
     final output multiply; sqrt via exp(0.5*ln). Dummy PE transposes during
     the DMA preamble and AllReduce waits keep the HAM clock at 2.4GHz.
"""

import os
import sys
import types

import numpy as np
import ml_dtypes

B = 64
N_IN = 1152
D_IN = 8
N_OUT = 64
D_OUT = 16
O_LOC = 8
N_CORES = 8
KD = N_IN * D_IN  # 9216, di-major: flat index = di*1152 + i
NCH = 9           # i chunks of 128
EPS = 1e-8

bf16 = ml_dtypes.bfloat16

_CACHE = {}
last_exec_ns = None

# how many of the 9 drain-pairs per g go through the fused DVE STT path
# (rest: ACT copy + DVE mul)
N_STT = 2
# s-matmul o-chain interleave width (1 = sequential, 2/4/8 = concurrent)
S_ILV_0 = 4   # iteration 0 (rhs = xt, no y tiles needed)
S_ILV = 2     # iterations 1, 2


def _install_ntff_hook():
    try:
        import antenv
    except ImportError:
        return
    if "antenv.axon_hooks" in sys.modules:
        return
    mod = types.ModuleType("antenv.axon_hooks")
    _state = {"hook": None}
    mod.set_axon_ntff_profile_hook = lambda h: _state.__setitem__("hook", h)
    mod.get_axon_ntff_profile_hook = lambda: _state["hook"]
    sys.modules["antenv.axon_hooks"] = mod
    antenv.axon_hooks = mod
    try:
        from trn_agent_boot.trn_boot import _ntff_profile_via_ctypes
        hook = _ntff_profile_via_ctypes("/opt/axon/libaxon_pjrt.so")
        if hook is not None:
            mod.set_axon_ntff_profile_hook(hook)
    except Exception:
        pass


def _build():
    import concourse.bacc as bacc
    import concourse.tile as tile
    import concourse.mybir as mybir

    dt = mybir.dt
    Alu = mybir.AluOpType
    Act = mybir.ActivationFunctionType

    nc = bacc.Bacc("TRN2", target_bir_lowering=False, debug=False,
                   num_devices=N_CORES)

    # ---- DRAM I/O ----
    d_xt = nc.dram_tensor("xt", [128, NCH, D_IN, B], dt.bfloat16,
                          kind="ExternalInput")
    d_wf = nc.dram_tensor("wf", [128, D_IN, NCH, O_LOC * D_OUT], dt.bfloat16,
                          kind="ExternalInput")
    d_wdo = nc.dram_tensor("wdo", [4, 16, 2, KD], dt.bfloat16,
                           kind="ExternalInput")
    d_xb = nc.dram_tensor("xb", [B, KD], dt.bfloat16, kind="ExternalInput")
    d_ones = nc.dram_tensor("onesbd", [128, 4], dt.float32,
                            kind="ExternalInput")
    d_idf = nc.dram_tensor("identf", [128, 128], dt.float32,
                           kind="ExternalInput")
    d_idb = nc.dram_tensor("identb", [128, 128], dt.bfloat16,
                           kind="ExternalInput")
    d_out = nc.dram_tensor("yout", [B, 2, 128], dt.float32,
                           kind="ExternalOutput")
    dbg = bool(int(os.environ.get("CAPS_DEBUG", "0")))
    if dbg:
        d_dbg_sT = nc.dram_tensor("dbg_sT", [128, 2, B], dt.bfloat16,
                                  kind="ExternalOutput")
        d_dbg_f = nc.dram_tensor("dbg_f", [128, 4], dt.float32,
                                 kind="ExternalOutput")
        d_dbg_pm = nc.dram_tensor("dbg_pm", [128, KD], dt.bfloat16,
                                  kind="ExternalOutput")
        d_dbg_eT = nc.dram_tensor("dbg_eT", [128, 2, NCH, B], dt.bfloat16,
                                  kind="ExternalOutput")
        d_dbg_z = nc.dram_tensor("dbg_z", [128, NCH, B], dt.bfloat16,
                                 kind="ExternalOutput")

    with tile.TileContext(nc) as tc:
        with (
            tc.tile_pool(name="const", bufs=1) as constp,
            tc.tile_pool(name="state", bufs=1) as statep,
            tc.tile_pool(name="work", bufs=1) as workp,
            tc.tile_pool(name="wk2", bufs=2) as workp2,
            tc.tile_pool(name="pswv", bufs=1, space="PSUM") as pswv,
            tc.tile_pool(name="pstr", bufs=2, space="PSUM") as pstr,
            tc.tile_pool(name="psS", bufs=1, space="PSUM") as psS,
            tc.tile_pool(name="tiny", bufs=1, space="PSUM") as tinyp,
            tc.tile_pool(name="dram", bufs=1, space="DRAM") as dramp,
        ):
            # ---------- constants / inputs ----------
            idf = constp.tile([128, 128], dt.float32)
            nc.sync.dma_start(idf[:], d_idf[:])
            xt = constp.tile([128, NCH, D_IN, B], dt.bfloat16)
            nc.sync.dma_start(xt[:], d_xt[:])
            wf = constp.tile([128, D_IN, NCH, O_LOC * D_OUT], dt.bfloat16)
            nc.sync.dma_start(wf[:], d_wf[:])
            onesbd = constp.tile([128, 4], dt.float32)
            nc.sync.dma_start(onesbd[:], d_ones[:])
            idb = constp.tile([128, 128], dt.bfloat16)
            nc.sync.dma_start(idb[:], d_idb[:])
            # it1-only inputs: DMAs issued after the it0 matmuls below
            wdo = constp.tile([128, 2, KD], dt.bfloat16)
            xb2 = constp.tile([128, KD], dt.bfloat16)

            # persistent state
            sT = statep.tile([128, 2, B], dt.bfloat16, tag="sT")
            sTf = statep.tile([128, 2, B], dt.float32, tag="sTf")
            fT2 = statep.tile([128, 4], dt.float32, tag="fT2")
            epst = statep.tile([4, 1], dt.float32, tag="epst")
            nc.gpsimd.memset(epst[:], EPS)
            beta = [statep.tile([128, N_IN], dt.float32, tag=f"beta{g}",
                                name=f"beta{g}") for g in range(4)]
            # eT[g][i, h, ic, b] = exp(beta)[o=(g,h)] transposed
            eT = [statep.tile([128, 2, NCH, B], dt.bfloat16, tag=f"eT{g}",
                              name=f"eT{g}") for g in range(4)]
            ZT = statep.tile([128, NCH, B], dt.bfloat16, tag="ZT")
            zsb = statep.tile([128, NCH, B], dt.bfloat16, tag="zsb")
            rzb = statep.tile([128, NCH, B], dt.bfloat16, tag="rzb")
            xz = statep.tile([128, NCH, D_IN, B], dt.bfloat16, tag="xz")
            zbA = statep.tile([128, NCH, B], dt.bfloat16, tag="zbA")
            zbB = statep.tile([128, NCH, B], dt.bfloat16, tag="zbB")
            zf = statep.tile([128, NCH, B], dt.float32, tag="zf")

            # beta-phase work tiles. Dependencies are tile-granular, so the
            # drain staging and PSUM accumulators are SPLIT into alternating
            # tiles to decouple wave jj+1's matmuls from wave jj's drain,
            # and pmul ping-pongs per g so the tree-adds of g overlap the
            # drains of g+1.
            pmuls = [workp.tile([128, KD], dt.bfloat16, tag=f"pmul{k}",
                                name=f"pmul{k}") for k in range(2)]
            pba = workp.tile([128, 1024], dt.bfloat16, tag="pba")
            pbb = workp.tile([128, 1024], dt.bfloat16, tag="pbb")
            pbufs = [pba, pbb]

            wva = pswv.tile([128, 2, 512], dt.float32, tag="wva")
            wvb = pswv.tile([128, 2, 512], dt.float32, tag="wvb")
            wvs = [wva, wvb]
            # one shared PSUM bank for all the small outputs
            tsh = tinyp.tile([128, 512], dt.float32, tag="tsh")

            # DRAM staging for the AllReduce (bf16 halves the wire bytes)
            zin = dramp.tile([128, NCH, B], dt.bfloat16, tag="zin")
            zout = dramp.tile([128, NCH, B], dt.bfloat16, tag="zout")
            zdin = dramp.tile([4, 1], dt.float32, tag="zdin")
            zdout = dramp.tile([4, 1], dt.float32, tag="zdout")
            # dummy collective: pays the mesh-setup / initial core-skew cost
            # during iteration 0 instead of inside the first real AllReduce
            nc.sync.dma_start(zdin[:], epst[:])
            nc.gpsimd.collective_compute(
                "AllReduce", Alu.add,
                ins=[zdin.opt()], outs=[zdout.opt()],
                replica_groups=[list(range(N_CORES))],
            )

            def squash(ps, first, last):
                scale = (1.0 / N_OUT) if first else 1.0
                sq = workp2.tile([128, 2, B], dt.float32, tag="sq")
                for h in range(2):
                    nc.vector.tensor_copy(sT[:, h, :], ps[:, h, :])
                    if last:
                        nc.vector.tensor_copy(sTf[:, h, :], ps[:, h, :])
                    nc.scalar.activation(sq[:, h, :], ps[:, h, :], Act.Square,
                                         scale=scale)
                n2ps = tsh[:4, :128].rearrange("p (h b) -> p h b", h=2)
                for h in range(2):
                    nc.tensor.matmul(n2ps[:, h, :], onesbd[:], sq[:, h, :],
                                     start=True, stop=True)
                n2 = workp2.tile([4, 2, B], dt.float32, tag="n2s")
                nc.vector.tensor_copy(n2[:], n2ps[:])
                # sqrt(n2+eps) = exp(0.5*ln(n2+eps)): Ln/Exp/Square/Copy all
                # live in ONE ACT table set -> no table switches anywhere
                lnn = workp2.tile([4, 2, B], dt.float32, tag="fl")
                nc.scalar.activation(lnn[:], n2[:], Act.Ln, bias=epst[:])
                srt = workp2.tile([4, 2, B], dt.float32, tag="fs")
                nc.scalar.activation(srt[:], lnn[:], Act.Exp, scale=0.5)
                a = workp2.tile([4, 2, B], dt.float32, tag="fa")
                # a = (n2 + 1) * sqrt(n2 + eps)
                nc.vector.scalar_tensor_tensor(a[:], n2[:], 1.0, srt[:],
                                               Alu.add, Alu.mult)
                r = workp2.tile([4, 2, B], dt.float32, tag="fr")
                nc.vector.reciprocal_approx_fast(
                    r.rearrange("g h b -> g (h b)"),
                    a.rearrange("g h b -> g (h b)"))
                f = workp2.tile([4, 2, B], dt.float32, tag="ff")
                nc.vector.tensor_mul(f[:], n2[:], r[:])
                if first:
                    nc.vector.tensor_scalar_mul(f[:], f[:], 1.0 / N_OUT)
                fps = tsh[:, 128:132]
                nc.tensor.transpose(
                    fps, f.rearrange("g h b -> g (h b)"), idf[:4, :4])
                nc.vector.tensor_copy(fT2[:], fps)

            def s_matmuls(ps, rhs_of, ilv):
                """o-chains interleaved in groups of `ilv` for PE col-tile
                concurrency; each o's accumulation order stays (ic, di)."""
                n_acc = NCH * D_IN
                for o0 in range(0, O_LOC, ilv):
                    for ic in range(NCH):
                        for di in range(D_IN):
                            k = ic * D_IN + di
                            for o in range(o0, o0 + ilv):
                                g, h = o % 4, o // 4
                                nc.tensor.matmul(
                                    ps[32 * g:32 * g + 16, h, :],
                                    wf[:, di, ic, 16 * o:16 * o + 16],
                                    rhs_of(o, ic, di),
                                    start=(k == 0), stop=(k == n_acc - 1),
                                    tile_position=(0, 32 * g),
                                )

            warm_n = [0]

            def warm(n):
                # dummy transposes: ramp/hold the PE HAM clock (idle >3.4us
                # re-throttles the PE to half clock)
                for _ in range(n):
                    trw = pstr.tile([128, 3, 128], dt.float32, tag="tr",
                                    name=f"warm{warm_n[0]}")
                    warm_n[0] += 1
                    nc.tensor.transpose(trw[:, 0, :], idf[:], idf[:])

            # ---------- iteration 0 ----------
            # ramp the PE clock while the input DMAs stream in
            warm(20)
            ps0 = psS.tile([128, 2, B], dt.float32, tag="sps")
            s_matmuls(ps0, lambda o, ic, di: xt[:, ic, di, :], S_ILV_0)
            # it1 inputs stream in while iteration 0 computes
            for g in range(4):
                nc.sync.dma_start(wdo[32 * g:32 * g + 16, :, :], d_wdo[g])
            nc.sync.dma_start(xb2[:B, :], d_xb[:])
            nc.sync.dma_start(xb2[B:, :], d_xb[:])
            warm(10)
            squash(ps0, first=True, last=False)
            if dbg:
                nc.sync.dma_start(d_dbg_sT[:], sT[:])
                nc.sync.dma_start(d_dbg_f[:], fT2[:])

            # ---------- iterations 1, 2 ----------
            for it in (1, 2):
                # ----- beta increments -----
                wave = 0
                for g in range(4):
                    pmul = pmuls[g % 2]
                    for jj in range(9):
                        wv = wvs[wave % 2]
                        wave += 1
                        for j2 in range(2):
                            j = 2 * jj + j2
                            for h in range(2):
                                nc.tensor.matmul(
                                    wv[64 * h:64 * h + 64, j2, :],
                                    sT[32 * g:32 * g + 16, h, :],
                                    wdo[32 * g:32 * g + 16, h,
                                        512 * j:512 * (j + 1)],
                                    start=True, stop=True,
                                    tile_position=(32 * g, 64 * h),
                                )
                        src = wv.rearrange("p a n -> p (a n)")
                        dst = pmul[:, 1024 * jj:1024 * (jj + 1)]
                        if jj >= 9 - N_STT:
                            nc.vector.scalar_tensor_tensor(
                                dst, src, fT2[:, g:g + 1],
                                xb2[:, 1024 * jj:1024 * (jj + 1)],
                                Alu.mult, Alu.mult)
                        else:
                            pdst = pbufs[jj % 2][:]
                            nc.scalar.activation(pdst, src, Act.Copy,
                                                 scale=fT2[:, g:g + 1])
                            nc.vector.tensor_mul(
                                dst, pdst,
                                xb2[:, 1024 * jj:1024 * (jj + 1)])
                    # di-major tree-adds: 5 bf16 pair-adds + 2 accumulations
                    pv = pmul.rearrange("p (d n) -> p d n", d=D_IN)
                    if it == 2:
                        binc = workp2.tile([128, N_IN], dt.float32,
                                           tag="binc", name=f"binc{g}")
                    tgt = beta[g] if it == 1 else binc
                    tqa = workp2.tile([128, N_IN], dt.bfloat16, tag="tqA",
                                      name=f"tqa{it}_{g}")
                    tqb = workp2.tile([128, N_IN], dt.bfloat16, tag="tqB",
                                      name=f"tqb{it}_{g}")
                    nc.vector.tensor_add(tqa[:], pv[:, 0, :], pv[:, 1, :])
                    nc.vector.tensor_add(tqb[:], pv[:, 2, :], pv[:, 3, :])
                    nc.vector.tensor_add(tgt[:], tqa[:], tqb[:])
                    tqa = workp2.tile([128, N_IN], dt.bfloat16, tag="tqA",
                                      name=f"tqa2{it}_{g}")
                    tqb = workp2.tile([128, N_IN], dt.bfloat16, tag="tqB",
                                      name=f"tqb2{it}_{g}")
                    nc.vector.tensor_add(tqa[:], pv[:, 4, :], pv[:, 5, :])
                    nc.vector.tensor_add(tqb[:], pv[:, 6, :], pv[:, 7, :])
                    nc.vector.tensor_add(tqa[:], tqa[:], tqb[:])
                    nc.vector.tensor_add(tgt[:], tgt[:], tqa[:])
                    if dbg and it == 1 and g == 0:
                        nc.sync.dma_start(d_dbg_pm[:], pmul[:])
                    # ----- transpose beta (+accum binc) + exp drain -----
                    for tt in range(3):
                        tr = pstr.tile([128, 3, 128], dt.float32, tag="tr",
                                       name=f"tr{it}_{g}_{tt}")
                        for u in range(3):
                            ic = 3 * tt + u
                            if it == 1:
                                nc.tensor.transpose(
                                    tr[:, u, :],
                                    beta[g][:, 128 * ic:128 * (ic + 1)],
                                    idf[:])
                            else:
                                nc.tensor.matmul(
                                    tr[:, u, :],
                                    beta[g][:, 128 * ic:128 * (ic + 1)],
                                    idf[:], is_transpose=True,
                                    start=True, stop=False)
                                nc.tensor.matmul(
                                    tr[:, u, :],
                                    binc[:, 128 * ic:128 * (ic + 1)],
                                    idf[:], is_transpose=True,
                                    start=False, stop=True)
                        for h in range(2):
                            nc.scalar.activation(
                                eT[g][:, h, 3 * tt:3 * tt + 3, :],
                                tr[:, :, 64 * h:64 * h + 64], Act.Exp)

                # ----- ZT = sum_o eT (bf16), AllReduce in bf16 -----
                nc.vector.tensor_add(zbA[:], eT[0][:, 0, :, :],
                                     eT[0][:, 1, :, :])
                nc.vector.tensor_add(zbA[:], zbA[:], eT[1][:, 0, :, :])
                nc.vector.tensor_add(zbA[:], zbA[:], eT[1][:, 1, :, :])
                nc.vector.tensor_add(zbB[:], eT[2][:, 0, :, :],
                                     eT[2][:, 1, :, :])
                nc.vector.tensor_add(zbB[:], zbB[:], eT[3][:, 0, :, :])
                nc.vector.tensor_add(zbB[:], zbB[:], eT[3][:, 1, :, :])
                nc.vector.tensor_add(ZT[:], zbA[:], zbB[:])

                nc.sync.dma_start(zin[:], ZT[:])
                nc.gpsimd.collective_compute(
                    "AllReduce", Alu.add,
                    ins=[zin.opt()], outs=[zout.opt()],
                    replica_groups=[list(range(N_CORES))],
                )
                nc.sync.dma_start(zsb[:], zout[:])

                # keep the PE warm through the AllReduce wait
                warm(24 if it == 1 else 12)

                if dbg and it == 1:
                    nc.sync.dma_start(d_dbg_eT[:], eT[0][:])
                    nc.sync.dma_start(d_dbg_z[:], zsb[:])

                # ----- rz, xz -----
                nc.vector.tensor_copy(zf[:], zsb[:])
                rzf = workp.tile([128, NCH, B], dt.float32, tag="rzf",
                                  name=f"rzf{it}")
                nc.vector.reciprocal_approx_fast(
                    rzf.rearrange("p a b -> p (a b)"),
                    zf.rearrange("p a b -> p (a b)"))
                nc.vector.tensor_copy(rzb[:], rzf[:])
                rbc = rzb.unsqueeze(2).broadcast_to([128, NCH, D_IN, B])
                nc.vector.tensor_mul(xz[:], xt[:], rbc)

                # ----- y + s-matmuls, o-pair interleaved -----
                psY = psS.tile([128, 2, B], dt.float32, tag="sps",
                               name=f"psY{it}")
                ytiles = {}

                def rhs_y(o, ic, di):
                    c = ic // 3
                    key = (o, c)
                    if key not in ytiles:
                        g, h = o % 4, o // 4
                        yb = workp2.tile([128, 3, D_IN, B], dt.bfloat16,
                                         tag=f"y{o % S_ILV}",
                                         name=f"y{it}_{c}_{o}")
                        ebc = eT[g][:, h, 3 * c:3 * c + 3, :]\
                            .unsqueeze(2).broadcast_to([128, 3, D_IN, B])
                        nc.vector.tensor_mul(
                            yb[:], xz[:, 3 * c:3 * c + 3, :, :], ebc)
                        ytiles[key] = yb
                    return ytiles[key][:, ic % 3, di, :]

                s_matmuls(psY, rhs_y, S_ILV)
                squash(psY, first=False, last=(it == 2))

            # ---------- final output ----------
            for h in range(2):
                op = tsh[:B, 256 + 128 * h:256 + 128 * (h + 1)]
                nc.tensor.transpose(op, sTf[:, h, :], idf[:])
                ofin = workp2.tile([B, 128], dt.float32, tag="ofin",
                                   name=f"ofin{h}")
                fbc = fT2[64 * h:64 * h + 64, :].unsqueeze(2).broadcast_to(
                    [B, 4, 32])
                nc.vector.tensor_mul(
                    ofin.rearrange("b (o r) -> b o r", o=4),
                    op.rearrange("b (o r) -> b o r", o=4),
                    fbc)
                nc.sync.dma_start(d_out[:, h, :], ofin[:])

    nc.compile()
    return nc


def _host_prep(x, W):
    xtc = np.ascontiguousarray(
        x.transpose(1, 2, 0).reshape(NCH, 128, D_IN, B)
        .transpose(1, 0, 2, 3).astype(bf16))
    xb = np.ascontiguousarray(x.transpose(0, 2, 1).reshape(B, KD).astype(bf16))
    onesbd = np.zeros((128, 4), np.float32)
    for g in range(4):
        onesbd[32 * g:32 * g + 16, g] = 1.0
    idf = np.eye(128, dtype=np.float32)

    in_maps = []
    for c in range(N_CORES):
        Wc = W[c * O_LOC:(c + 1) * O_LOC]
        wfc = np.ascontiguousarray(
            Wc.transpose(1, 3, 0, 2)
            .reshape(NCH, 128, D_IN, O_LOC * D_OUT)
            .transpose(1, 2, 0, 3).astype(bf16))
        wdoc = np.zeros((4, 16, 2, KD), np.float32)
        for g in range(4):
            for h in range(2):
                o = 4 * h + g
                # [do, (di, i)] di-major
                wdoc[g, :, h, :] = (
                    Wc[o].transpose(1, 2, 0).reshape(D_OUT, KD))
        in_maps.append({
            "xt": xtc, "wf": wfc,
            "wdo": np.ascontiguousarray(wdoc.astype(bf16)),
            "xb": xb, "onesbd": onesbd, "identf": idf,
            "identb": idf.astype(bf16),
        })
    return in_maps


def kernel(input, W):
    global last_exec_ns
    _install_ntff_hook()
    from concourse.bass_utils import run_bass_kernel_spmd

    x = np.asarray(input, dtype=np.float32)
    W = np.asarray(W, dtype=np.float32)

    if "nc" not in _CACHE:
        _CACHE["nc"] = _build()
    nc = _CACHE["nc"]

    in_maps = _host_prep(x, W)
    trace = bool(int(os.environ.get("CAPS_TRACE", "0")))
    res = run_bass_kernel_spmd(nc, in_maps, core_ids=list(range(N_CORES)),
                               trace=trace)
    last_exec_ns = res.exec_time_ns
    _CACHE["res"] = res

    outs = []
    for c in range(N_CORES):
        y = res.results[c]["yout"].reshape(B, 2, 4, 32)[:, :, :, :16]
        outs.append(y.reshape(B, 8, D_OUT))
    return np.concatenate(outs, axis=1).astype(np.float32)
